# revision 6
# baseline (speedup 1.0000x reference)
"""Varlen causal GQA attention on 8 TRN2 NeuronCores.

Sharding: tensor-parallel over heads. Core c gets KV head c and its 4
query heads (GQA group), so every core runs an identical program on its
own head-slice of q/k/v and produces its own head-slice of the output.
No cross-core communication.

Per core, per (sequence, head, 128-row query tile):
  - S^T tile [kv, q] = matmul(lhsT=K^T, rhs=Q^T) in PSUM (bf16 in, f32 acc)
  - A^T = exp(SCALE * S^T) via ScalarE -> bf16 SBUF (no max subtraction:
    logits are O(1) so exp is safe in f32/bf16 range)
  - causal mask on the diagonal tile via a 0/1 triangular multiply
  - O tile [q, d | rowsum] = sum_j matmul(lhsT=A^T_j, rhs=[V_j | ones])
    accumulated in PSUM; the extra ones column yields the softmax
    denominator in the same matmul.
  - normalize with reciprocal(rowsum) and DMA out.

Q^T/K^T are produced with PE transposes (f32) + cast-to-bf16 copies.
"""

import os
import sys

import numpy as np

for _p in ("/opt/trn_rl_repo", "/root/.axon_site/_ro/trn_rl_repo"):
    if os.path.isdir(_p) and _p not in sys.path:
        sys.path.insert(0, _p)

NUM_HEADS = 32
NUM_KV_HEADS = 8
HEAD_DIM = 128
SCALE = 0.08838834764831845  # head_dim ** -0.5
N_CORES = 8
HPC = NUM_HEADS // N_CORES  # q heads per core = 4
DQ = HPC * HEAD_DIM  # 512

_BUILD_CACHE = {}
LAST_RESULT = None

# The walrus in this image only encodes 2 sem-waits per instruction; Tile's
# kernel-tail drain accumulates one wait per live semaphore. Split it into a
# chain of drains, each carrying at most 2 waits.
_MAX_WAITS = 1
_drain_patched = False


def _patch_tile_drain():
    global _drain_patched
    if _drain_patched:
        return
    import concourse.tile as tile
    from concourse import mybir
    from concourse.vector_clock import ScopedClock

    def _drain_and_barrier(self, tick_clock, wait_clock):
        nc = self.nc
        drain_inst = nc.sync.drain()
        wait_clock.add_sem_waits(
            drain_inst.ins, ScopedClock({None: tick_clock.global_clock})
        )
        si = drain_inst.ins.sync_info
        waits = list(si.on_wait) if si is not None and si.on_wait else []
        if len(waits) > _MAX_WAITS:
            drain_inst.ins.sync_info = mybir.SyncInfo(
                on_wait=waits[:_MAX_WAITS],
                on_update=list(si.on_update) if si.on_update else [],
            )
            for i in range(_MAX_WAITS, len(waits), _MAX_WAITS):
                extra = nc.sync.drain()
                extra.ins.sync_info = mybir.SyncInfo(
                    on_wait=waits[i : i + _MAX_WAITS], on_update=[]
                )
        nc.all_engine_barrier()
        assert self.sems is not None
        popped = nc._tile_sem_poison_stack.pop()
        assert popped is self._sem_poison
        nc.clear_and_free_semaphores(list(self.sems.allocated().values()))
        nc.all_engine_barrier()

    tile.TileContext._drain_and_barrier = _drain_and_barrier
    _drain_patched = True


def _split_excess_waits(nc):
    """The walrus in this image encodes at most 1 sem-wait per instruction
    (2 for Drain). Tile emits up to ~3. Hoist excess waits onto standalone
    EventSemaphore carriers on the same engine, inserted just before the
    over-limit instruction (same-engine program order preserves semantics).
    """
    from concourse import mybir

    n = 0
    for bb in nc.main_func.blocks:
        out = []
        for ins in bb.instructions:
            si = getattr(ins, "sync_info", None)
            waits = list(si.on_wait) if si is not None and si.on_wait else []
            limit = 1
            if len(waits) > limit:
                for w in waits[:-limit]:
                    n += 1
                    out.append(
                        mybir.InstEventSemaphore(
                            name=f"WSPLIT-{n}",
                            engine=ins.engine,
                            sync_info=mybir.SyncInfo(on_wait=[w], on_update=[]),
                            ins=[],
                            outs=[],
                        )
                    )
                ins.sync_info = mybir.SyncInfo(
                    on_wait=waits[-limit:],
                    on_update=list(si.on_update) if si.on_update else [],
                )
            out.append(ins)
        bb.instructions[:] = out
    return n


def _build(lens):
    import concourse.bass as bass
    import concourse.tile as tile
    from concourse import mybir
    from concourse.bass import ds, ts
    from concourse.masks import make_identity

    _patch_tile_drain()

    f32 = mybir.dt.float32
    bf16 = mybir.dt.bfloat16
    T = int(sum(lens))

    nc = bass.Bass()
    q_d = nc.declare_dram_parameter("q", [T, DQ], f32, isOutput=False)
    k_d = nc.declare_dram_parameter("k", [T, HEAD_DIM], f32, isOutput=False)
    v_d = nc.declare_dram_parameter("v", [T, HEAD_DIM], f32, isOutput=False)
    o_d = nc.declare_dram_parameter("out", [T, DQ], f32, isOutput=True)

    with tile.TileContext(nc) as tc:
        with (
            tc.tile_pool(name="consts", bufs=1) as consts,
            tc.tile_pool(name="kvseq", bufs=2) as kvseq,
            tc.tile_pool(name="work", bufs=3) as work,
            tc.tile_pool(name="qtp", bufs=6) as qtp,
            tc.tile_pool(name="aexp", bufs=4) as aexp,
            tc.tile_pool(name="ps_t", bufs=2, space="PSUM") as ps_t,
            tc.tile_pool(name="ps_s", bufs=3, space="PSUM") as ps_s,
            tc.tile_pool(name="ps_o", bufs=2, space="PSUM") as ps_o,
        ):
            ident = consts.tile([128, 128], f32)
            make_identity(nc, ident)
            # tri[p, f] = 1 if f >= p else 0  (keep q_pos >= kv_pos on the
            # diagonal tile of S^T, where partitions=kv and free=q)
            tri = consts.tile([128, 128], bf16)
            nc.gpsimd.memset(tri, 1.0)
            nc.gpsimd.affine_select(
                out=tri,
                in_=tri,
                compare_op=mybir.AluOpType.is_ge,
                fill=0.0,
                base=0,
                pattern=[[1, 128]],
                channel_multiplier=-1,
            )

            off = 0
            for L in lens:
                L = int(L)
                nt = (L + 127) // 128
                nfull = L // 128
                rrem = L - nfull * 128

                # ---- K: load natural layout, PE-transpose to K^T bf16 ----
                k_nat = kvseq.tile([128, 8, 128], f32, tag="k_nat")
                if nfull:
                    nc.sync.dma_start(
                        out=k_nat[:, 0:nfull, :],
                        in_=k_d[off : off + nfull * 128, :].rearrange(
                            "(t p) d -> p t d", p=128
                        ),
                    )
                if rrem:
                    nc.sync.dma_start(
                        out=k_nat[:rrem, nfull, :],
                        in_=k_d[off + nfull * 128 : off + L, :],
                    )
                kt = kvseq.tile([128, 8 * 128], bf16, tag="kt")
                for j in range(nt):
                    jr = 128 if j < nfull else rrem
                    tp = ps_t.tile([128, 128], f32, tag="tp")
                    nc.tensor.transpose(
                        tp[:, :jr], k_nat[:jr, j, :], ident[:jr, :jr]
                    )
                    nc.vector.tensor_copy(kt[:, ds(j * 128, jr)], tp[:, :jr])

                # ---- V: load natural layout, cast to bf16, append ones col ----
                v_nat = kvseq.tile([128, 8, 128], f32, tag="v_nat")
                if nfull:
                    nc.sync.dma_start(
                        out=v_nat[:, 0:nfull, :],
                        in_=v_d[off : off + nfull * 128, :].rearrange(
                            "(t p) d -> p t d", p=128
                        ),
                    )
                if rrem:
                    nc.sync.dma_start(
                        out=v_nat[:rrem, nfull, :],
                        in_=v_d[off + nfull * 128 : off + L, :],
                    )
                v_sb = kvseq.tile([128, 8, 136], bf16, tag="v_sb")
                if nfull:
                    nc.vector.tensor_copy(
                        v_sb[:, 0:nfull, 0:128], v_nat[:, 0:nfull, :]
                    )
                if rrem:
                    nc.vector.tensor_copy(
                        v_sb[:rrem, nfull, 0:128], v_nat[:rrem, nfull, :]
                    )
                nc.vector.memset(v_sb[:, 0:nt, 128:129], 1.0)

                # ---- main attention loops ----
                for i in range(nt):
                    ir = 128 if i < nfull else rrem
                    row0 = off + i * 128
                    q_nat = work.tile([128, DQ], f32, tag="q_nat")
                    nc.sync.dma_start(
                        out=q_nat[:ir, :], in_=q_d[row0 : row0 + ir, :]
                    )
                    out_sb = work.tile([128, DQ], f32, tag="out_sb")
                    for h in range(HPC):
                        tp = ps_t.tile([128, 128], f32, tag="tp")
                        nc.tensor.transpose(
                            tp[:, :ir], q_nat[:ir, ts(h, 128)], ident[:ir, :ir]
                        )
                        qt = qtp.tile([128, 128], bf16, tag="qt")
                        nc.vector.tensor_copy(qt[:, :ir], tp[:, :ir])

                        o_ps = ps_o.tile([128, 129], f32, tag="o_ps")
                        for j in range(i + 1):
                            jr = 128 if j < nfull else rrem
                            s_ps = ps_s.tile([128, 128], f32, tag="s_ps")
                            nc.tensor.matmul(
                                s_ps[:jr, :ir],
                                kt[:, ds(j * 128, jr)],
                                qt[:, :ir],
                            )
                            a_sb = aexp.tile([128, 128], bf16, tag="a_sb")
                            nc.scalar.activation(
                                out=a_sb[:jr, :ir],
                                in_=s_ps[:jr, :ir],
                                func=mybir.ActivationFunctionType.Exp,
                                scale=SCALE,
                            )
                            if j == i:
                                nc.vector.tensor_mul(
                                    a_sb[:jr, :ir], a_sb[:jr, :ir], tri[:jr, :ir]
                                )
                            nc.tensor.matmul(
                                o_ps[:ir, :],
                                a_sb[:jr, :ir],
                                v_sb[:jr, j, 0:129],
                                start=(j == 0),
                                stop=(j == i),
                            )
                        recip = work.tile([128, 1], f32, tag="recip")
                        nc.vector.reciprocal(recip[:ir], o_ps[:ir, 128:129])
                        nc.vector.tensor_scalar_mul(
                            out_sb[:ir, ts(h, 128)], o_ps[:ir, 0:128], recip[:ir]
                        )
                    nc.sync.dma_start(
                        out=o_d[row0 : row0 + ir, :], in_=out_sb[:ir, :]
                    )
                off += L
    _split_excess_waits(nc)
    return nc


def _get_program(lens):
    key = tuple(int(x) for x in lens)
    if key not in _BUILD_CACHE:
        _BUILD_CACHE[key] = _build(key)
    return _BUILD_CACHE[key]


def kernel(q, k, v, cu_seqlens, max_seqlen=None, **_unused):
    global LAST_RESULT
    from concourse.bass_utils import run_bass_kernel_spmd

    q = np.ascontiguousarray(np.asarray(q, dtype=np.float32))
    k = np.ascontiguousarray(np.asarray(k, dtype=np.float32))
    v = np.ascontiguousarray(np.asarray(v, dtype=np.float32))
    cu = np.asarray(cu_seqlens).astype(np.int64)
    lens = tuple(int(cu[i + 1] - cu[i]) for i in range(len(cu) - 1))
    T = int(cu[-1])
    assert q.shape == (T, NUM_HEADS * HEAD_DIM)

    nc = _get_program(lens)

    in_maps = []
    for c in range(N_CORES):
        in_maps.append(
            {
                "q": np.ascontiguousarray(q[:, c * DQ : (c + 1) * DQ]),
                "k": np.ascontiguousarray(
                    k[:, c * HEAD_DIM : (c + 1) * HEAD_DIM]
                ),
                "v": np.ascontiguousarray(
                    v[:, c * HEAD_DIM : (c + 1) * HEAD_DIM]
                ),
            }
        )

    trace = bool(int(os.environ.get("KERNEL_TRACE", "0")))
    LAST_RESULT = run_bass_kernel_spmd(
        nc, in_maps, core_ids=list(range(N_CORES)), trace=trace
    )
    out = np.concatenate(
        [LAST_RESULT.results[c]["out"] for c in range(N_CORES)], axis=1
    )
    return out.reshape(T, NUM_HEADS, HEAD_DIM).astype(np.float32)
